# revision 12
# baseline (speedup 1.0000x reference)
"""Trainium2 Bass kernel for NSETransformer forward (SE3 voxel flow + 3D grid_sample).

Problem: src (1,96,128,160,2) f32 volume; flow (N,7) per-voxel SE3 [t, q].
  N = 96*128*160 = 1,966,080 voxels, C = 2 channels.
Outputs (matching reference):
  warped  (1,2,96,128,160)  trilinear border-clamped sample of src at SE3-moved points
  new_loc (1,3,96,128,160)  the SE3-transformed grid points
  grid    (1,3,96,128,160)  constant normalized grid

Sharding: flattened voxel dim split evenly across 8 cores (245,760 voxels each);
src replicated to every core's HBM; no cross-core communication.

Per-core kernel: chunks of 61,440 voxels ([128 partitions x 480 cols]).
SE3 math + sampling coords + trilinear weights on DVE/ACT; the 8-corner fetch is
4 indirect-DMA gathers per voxel chunk (one per (z,y) corner pair), each
descriptor pulling 16B = cells (z,y,x0) and (z,y,x0+1) = 4 floats (2 cells x C).
"""

import numpy as np

import concourse.bass as bass
import concourse.bacc as bacc
import concourse.mybir as mybir
from concourse import tile
from concourse.bass_utils import run_bass_kernel_spmd

D, H, W = 96, 128, 160
C = 2
N = D * H * W                 # 1,966,080
NCORES = 8
TPC = N // NCORES             # 245,760 voxels per core
P = 128                       # partitions
COLS = TPC // P               # 1920 columns per partition
CK = 480                      # chunk columns
NCHUNK = COLS // CK           # 4 chunks
PAD = 8                       # extra cells at end of src copy (last x-pair read)

MX = max(D, H, W) - 1         # 159
HW_ = H * W                   # 20480 cells per z-slice
F32 = mybir.dt.float32
I32 = mybir.dt.int32

_CACHE = {}


def _make_grid_np():
    # Matches reference _make_grid (f32 throughout)
    vs = [np.arange(s, dtype=np.float32) for s in (D, H, W)]
    g = np.stack(np.meshgrid(*vs, indexing="ij")).astype(np.float32)  # (3,D,H,W)
    for i, s in enumerate((D, H, W)):
        g[i] = 2.0 * (g[i] / (s - 1) - 0.5) * (s - 1) / MX
    return g


def _build_program():
    nc = bacc.Bacc("TRN2", target_bir_lowering=False, debug=False, num_devices=NCORES)

    src_t = nc.dram_tensor("src", (N + PAD, C), F32, kind="ExternalInput")
    flow_t = nc.dram_tensor("flow7", (7, TPC), F32, kind="ExternalInput")
    grid_t = nc.dram_tensor("grid3", (3, TPC), F32, kind="ExternalInput")
    warp_t = nc.dram_tensor("warped", (C, TPC), F32, kind="ExternalOutput")
    nloc_t = nc.dram_tensor("newloc", (3, TPC), F32, kind="ExternalOutput")
    # Expanded corner volume: E[cell] = 16 floats = v(z+pz, min-clamped y+py, x+dx, c)
    # for (pair=(pz,py), dx, c); one 64B gather per voxel fetches all 8 corners x C.
    e_t = nc.dram_tensor("evol", (N + PAD, 16), F32, kind="Internal")

    flow_v = flow_t.ap().rearrange("c (p j) -> c p j", p=P)
    grid_v = grid_t.ap().rearrange("c (p j) -> c p j", p=P)
    warp_v = warp_t.ap().rearrange("c (p j) -> c p j", p=P)
    nloc_v = nloc_t.ap().rearrange("c (p j) -> c p j", p=P)
    src_v = src_t.ap()[:N].rearrange("(d h w) c -> d h (w c)", d=D, h=H)
    e_v = e_t.ap()[:N].rearrange("(d h w) k -> d h (w k)", d=D, h=H)

    A = mybir.AluOpType
    ACTF = mybir.ActivationFunctionType

    with tile.TileContext(nc) as tc:
        # ---- Phase 0: build E volume (per z-slice; partition dim = y) ----
        with tc.tile_pool(name="eb", bufs=4) as eb:
            WC = W * C

            def load_slab_pair(zz, z):
                s0 = eb.tile([P, WC], F32, tag="slab0", name=f"s0_{z}_{zz}")
                nc.sync.dma_start(out=s0[:], in_=src_v[zz])
                s1 = eb.tile([P, WC], F32, tag="slab1", name=f"s1_{z}_{zz}")
                nc.sync.dma_start(out=s1[:P - 1], in_=src_v[zz, 1:P])
                nc.sync.dma_start(out=s1[P - 1:P], in_=src_v[zz, P - 1:P])
                return s0, s1

            prev = None
            for z in range(D):
                cur = prev if prev is not None else load_slab_pair(z, z)
                nxt = cur if z == D - 1 else load_slab_pair(z + 1, z)
                prev = nxt
                slabs = {"z0y0": cur[0], "z0y1": cur[1],
                         "z1y0": nxt[0], "z1y1": nxt[1]}
                ez = eb.tile([P, W * 16], F32, tag="ez", name=f"ez_{z}")
                ez3 = ez[:].rearrange("p (w k) -> p w k", k=16)
                # x = W-1 dx=1 slots must be finite (weight-0 garbage otherwise NaN-risk)
                nc.vector.memset(ez3[:, W - 1: W, :], 0.0)
                for pi, pair in enumerate(("z0y0", "z0y1", "z1y0", "z1y1")):
                    s = slabs[pair]
                    s3 = s[:].rearrange("p (w c) -> p w c", c=C)
                    # both channels per op, on ACT to keep DVE free for voxel math
                    # dx = 0 plane: all W
                    nc.scalar.activation(
                        ez3[:, :, pi * 4: pi * 4 + 2], s3[:, :, :], ACTF.Copy)
                    # dx = 1 plane: first W-1 (x = W-1 slot left zero)
                    nc.scalar.activation(
                        ez3[:, : W - 1, pi * 4 + 2: pi * 4 + 4], s3[:, 1:, :], ACTF.Copy)
                # store on the ACT HWDGE ring so load/store issue streams parallelize
                nc.scalar.dma_start(out=e_v[z], in_=ez[:])

        with tc.tile_pool(name="io", bufs=2) as io, tc.tile_pool(name="tmp", bufs=1) as tp:
            for ch in range(NCHUNK):
                j0 = ch * CK
                sl = slice(j0, j0 + CK)

                def lt(pool, name, dt=F32, cols=CK):
                    return pool.tile([P, cols], dt, tag=name, name=f"{name}_c{ch}")

                # ---- loads ----
                px, py, pz = (lt(io, f"g{c}") for c in range(3))
                for t, c in ((px, 0), (py, 1), (pz, 2)):
                    nc.sync.dma_start(out=t[:], in_=grid_v[c, :, sl])
                fl = [lt(io, f"f{c}") for c in range(7)]
                for c in range(7):
                    nc.sync.dma_start(out=fl[c][:], in_=flow_v[c, :, sl])
                tx, ty, tz, qx, qy, qz, qw = fl

                # ---- SE3: uv = qv x p + qw*p ; n = p + 2*(qv x uv) + t ----
                m1 = lt(tp, "m1"); m2 = lt(tp, "m2")
                uv = [lt(tp, f"uv{c}") for c in range(3)]
                pcomp = (px, py, pz)
                qv = (qx, qy, qz)
                for c in range(3):
                    a, b = (c + 1) % 3, (c + 2) % 3
                    nc.vector.tensor_mul(m1[:], qv[a][:], pcomp[b][:])
                    nc.vector.tensor_mul(m2[:], qv[b][:], pcomp[a][:])
                    nc.vector.tensor_sub(uv[c][:], m1[:], m2[:])
                    nc.vector.tensor_mul(m1[:], qw[:], pcomp[c][:])
                    nc.vector.tensor_add(uv[c][:], uv[c][:], m1[:])
                nlocs = [lt(io, f"n{c}") for c in range(3)]
                tcomp = (tx, ty, tz)
                for c in range(3):
                    a, b = (c + 1) % 3, (c + 2) % 3
                    nc.vector.tensor_mul(m1[:], qv[a][:], uv[b][:])
                    nc.vector.tensor_mul(m2[:], qv[b][:], uv[a][:])
                    nc.vector.tensor_sub(m1[:], m1[:], m2[:])
                    nc.vector.scalar_tensor_tensor(
                        out=m1[:], in0=m1[:], scalar=2.0, in1=pcomp[c][:],
                        op0=A.mult, op1=A.add)
                    nc.vector.tensor_add(nlocs[c][:], m1[:], tcomp[c][:])
                    nc.sync.dma_start(out=nloc_v[c, :, sl], in_=nlocs[c][:])

                # ---- sampling coords per axis: i = clip(79.5*n + c0, 0, L-1) ----
                # floor via round-to-nearest(i - 0.5)  (cast rounds nearest-even;
                # exact-odd-integer i gives floor = i-1 with w = 1.0 -> still an
                # exact, in-bounds lerp)
                cf = []; wfrac = []
                for c, (c0, L) in enumerate(((47.5, D), (63.5, H), (79.5, W))):
                    ic = lt(tp, f"ic{c}")
                    nc.vector.tensor_scalar(
                        out=ic[:], in0=nlocs[c][:], scalar1=79.5, scalar2=c0,
                        op0=A.mult, op1=A.add)
                    nc.vector.tensor_scalar(
                        out=ic[:], in0=ic[:], scalar1=0.0, scalar2=float(L - 1),
                        op0=A.max, op1=A.min)
                    ci = lt(tp, f"ci{c}", I32)
                    nc.scalar.activation(ci[:], ic[:], ACTF.Copy, bias=-0.5)
                    cfc = lt(tp, f"cf{c}")
                    nc.scalar.activation(cfc[:], ci[:], ACTF.Copy)
                    wc = lt(tp, f"w{c}")
                    nc.vector.tensor_sub(wc[:], ic[:], cfc[:])
                    cf.append(cfc); wfrac.append(wc)
                cfd, cfh, cfw = cf
                wz, wy, wx = wfrac

                # ---- cell offset e(z,y,x) = z*HW_ + y*W + x (f32-exact) ----
                e00 = lt(tp, "e00")
                nc.vector.scalar_tensor_tensor(
                    out=e00[:], in0=cfd[:], scalar=float(HW_), in1=cfw[:],
                    op0=A.mult, op1=A.add)
                nc.vector.scalar_tensor_tensor(
                    out=e00[:], in0=cfh[:], scalar=float(W), in1=e00[:],
                    op0=A.mult, op1=A.add)

                # ---- gather: one 64B fetch per voxel from E (all 8 corners x C) ----
                # walrus indirect DMA consumes ONE offset per partition per call.
                oi = lt(io, "O00", I32)
                nc.scalar.activation(oi[:], e00[:], ACTF.Copy)
                G = io.tile([P, CK, 16], F32, tag="G", name=f"G_c{ch}")
                for j in range(CK):
                    nc.gpsimd.indirect_dma_start(
                        out=G[:, j, :], out_offset=None, in_=e_t.ap(),
                        in_offset=bass.IndirectOffsetOnAxis(ap=oi[:, j:j + 1], axis=0))

                # ---- trilinear: lerp x within each corner pair, then y, then z ----
                sx = [lt(tp, f"sx{i}") for i in range(8)]   # [pair][ch]
                for gi in range(4):
                    for c in range(2):
                        s = sx[gi * 2 + c]
                        nc.vector.tensor_sub(s[:], G[:, :, gi * 4 + 2 + c], G[:, :, gi * 4 + c])
                        nc.vector.tensor_mul(s[:], s[:], wx[:])
                        nc.vector.tensor_add(s[:], s[:], G[:, :, gi * 4 + c])
                vy = [lt(tp, f"vy{i}") for i in range(4)]   # [zlev][ch]
                for zi in range(2):
                    for c in range(2):
                        a = sx[zi * 4 + c]          # (z, y0, ch)
                        b = sx[zi * 4 + 2 + c]      # (z, y1, ch)
                        v = vy[zi * 2 + c]
                        nc.vector.tensor_sub(v[:], b[:], a[:])
                        nc.vector.tensor_mul(v[:], v[:], wy[:])
                        nc.vector.tensor_add(v[:], v[:], a[:])
                for c in range(2):
                    o = lt(io, f"out{c}")
                    nc.vector.tensor_sub(o[:], vy[2 + c][:], vy[c][:])
                    nc.vector.tensor_mul(o[:], o[:], wz[:])
                    nc.vector.tensor_add(o[:], o[:], vy[c][:])
                    nc.sync.dma_start(out=warp_v[c, :, sl], in_=o[:])

    nc.compile()
    return nc


def _get_program():
    if "nc" not in _CACHE:
        _CACHE["nc"] = _build_program()
    return _CACHE["nc"]


def _make_in_maps(src, flow):
    src = np.ascontiguousarray(np.asarray(src, dtype=np.float32))
    flow = np.ascontiguousarray(np.asarray(flow, dtype=np.float32))
    assert src.shape == (1, D, H, W, C) and flow.shape == (N, 7)

    grid = _make_grid_np()                                # (3,D,H,W) f32
    pts3 = grid.reshape(3, N)                             # component-major points

    src_cells = src.reshape(N, C)
    src_pad = np.concatenate([src_cells, np.zeros((PAD, C), np.float32)], axis=0)
    flow7 = np.ascontiguousarray(flow.T)                  # (7, N)

    in_maps = []
    for k in range(NCORES):
        sl = slice(k * TPC, (k + 1) * TPC)
        in_maps.append({
            "src": src_pad,
            "flow7": np.ascontiguousarray(flow7[:, sl]),
            "grid3": np.ascontiguousarray(pts3[:, sl]),
        })
    return in_maps


def kernel(src, flow):
    nc = _get_program()
    in_maps = _make_in_maps(src, flow)
    grid = _make_grid_np()

    res = run_bass_kernel_spmd(nc, in_maps, core_ids=list(range(NCORES)))
    warped = np.concatenate([res.results[k]["warped"] for k in range(NCORES)], axis=1)
    newloc = np.concatenate([res.results[k]["newloc"] for k in range(NCORES)], axis=1)

    warped = warped.reshape(1, C, D, H, W)
    newloc = newloc.reshape(1, 3, D, H, W)
    return warped, newloc, grid[None]


# revision 16
# speedup vs baseline: 1.0450x; 1.0450x over previous
"""Trainium2 Bass kernel for NSETransformer forward (SE3 voxel flow + 3D grid_sample).

Problem: src (1,96,128,160,2) f32 volume; flow (N,7) per-voxel SE3 [t, q].
  N = 96*128*160 = 1,966,080 voxels, C = 2 channels.
Outputs (matching reference):
  warped  (1,2,96,128,160)  trilinear border-clamped sample of src at SE3-moved points
  new_loc (1,3,96,128,160)  the SE3-transformed grid points
  grid    (1,3,96,128,160)  constant normalized grid

Sharding: flattened voxel dim split evenly across 8 cores (245,760 voxels each);
src replicated to every core's HBM; no cross-core communication.

Per-core kernel: chunks of 61,440 voxels ([128 partitions x 480 cols]).
SE3 math + sampling coords + trilinear weights on DVE/ACT; the 8-corner fetch is
4 indirect-DMA gathers per voxel chunk (one per (z,y) corner pair), each
descriptor pulling 16B = cells (z,y,x0) and (z,y,x0+1) = 4 floats (2 cells x C).
"""

import numpy as np

import concourse.bass as bass
import concourse.bacc as bacc
import concourse.mybir as mybir
from concourse import tile
from concourse.bass_utils import run_bass_kernel_spmd

D, H, W = 96, 128, 160
C = 2
N = D * H * W                 # 1,966,080
NCORES = 8
TPC = N // NCORES             # 245,760 voxels per core
P = 128                       # partitions
COLS = TPC // P               # 1920 columns per partition
CK = 480                      # chunk columns
NCHUNK = COLS // CK           # 4 chunks
PAD = 8                       # extra cells at end of src copy (last x-pair read)

MX = max(D, H, W) - 1         # 159
HW_ = H * W                   # 20480 cells per z-slice
F32 = mybir.dt.float32
I32 = mybir.dt.int32

_CACHE = {}


def _make_grid_np():
    # Matches reference _make_grid (f32 throughout)
    vs = [np.arange(s, dtype=np.float32) for s in (D, H, W)]
    g = np.stack(np.meshgrid(*vs, indexing="ij")).astype(np.float32)  # (3,D,H,W)
    for i, s in enumerate((D, H, W)):
        g[i] = 2.0 * (g[i] / (s - 1) - 0.5) * (s - 1) / MX
    return g


def _build_program():
    nc = bacc.Bacc("TRN2", target_bir_lowering=False, debug=False, num_devices=NCORES)

    src_t = nc.dram_tensor("src", (N + PAD, C), F32, kind="ExternalInput")
    flow_t = nc.dram_tensor("flow7", (7, TPC), F32, kind="ExternalInput")
    grid_t = nc.dram_tensor("grid3", (3, TPC), F32, kind="ExternalInput")
    warp_t = nc.dram_tensor("warped", (C, TPC), F32, kind="ExternalOutput")
    nloc_t = nc.dram_tensor("newloc", (3, TPC), F32, kind="ExternalOutput")
    # Expanded corner volume: E[cell] = 8 floats = v(z+pz, min-clamped y+py, x, c)
    # for (pair=(pz,py), c). One 64B gather per voxel fetches entries (cell, cell+1)
    # = all 8 corners x C (the x+1 corners are simply the next entry).
    e_t = nc.dram_tensor("evol", (N + PAD, 8), F32, kind="Internal")

    flow_v = flow_t.ap().rearrange("c (p j) -> c p j", p=P)
    grid_v = grid_t.ap().rearrange("c (p j) -> c p j", p=P)
    warp_v = warp_t.ap().rearrange("c (p j) -> c p j", p=P)
    nloc_v = nloc_t.ap().rearrange("c (p j) -> c p j", p=P)
    src_v = src_t.ap()[:N].rearrange("(d h w) c -> d h (w c)", d=D, h=H)
    e_v = e_t.ap()[:N].rearrange("(d h w) k -> d h (w k)", d=D, h=H)
    e_pad_v = e_t.ap()[N:N + 2]

    A = mybir.AluOpType
    ACTF = mybir.ActivationFunctionType

    with tile.TileContext(nc) as tc:
        # ---- Phase 0: build E volume (per z-slice; partition dim = y) ----
        with tc.tile_pool(name="eb", bufs=4) as eb:
            WC = W * C

            def load_slab_pair(zz, z):
                s0 = eb.tile([P, WC], F32, tag="slab0", name=f"s0_{z}_{zz}")
                nc.sync.dma_start(out=s0[:], in_=src_v[zz])
                s1 = eb.tile([P, WC], F32, tag="slab1", name=f"s1_{z}_{zz}")
                nc.sync.dma_start(out=s1[:P - 1], in_=src_v[zz, 1:P])
                nc.sync.dma_start(out=s1[P - 1:P], in_=src_v[zz, P - 1:P])
                return s0, s1

            # pad rows N..N+1 must be finite (last cell's +1 fetch reads row N)
            zpad = eb.tile([1, 16], F32, tag="zpad", name="zpad")
            nc.vector.memset(zpad[:], 0.0)
            nc.sync.dma_start(out=e_pad_v, in_=zpad[:])

            prev = None
            for z in range(D):
                cur = prev if prev is not None else load_slab_pair(z, z)
                nxt = cur if z == D - 1 else load_slab_pair(z + 1, z)
                prev = nxt
                slabs = {"z0y0": cur[0], "z0y1": cur[1],
                         "z1y0": nxt[0], "z1y1": nxt[1]}
                ez = eb.tile([P, W * 8], F32, tag="ez", name=f"ez_{z}")
                ez3 = ez[:].rearrange("p (w k) -> p w k", k=8)
                for pi, pair in enumerate(("z0y0", "z0y1", "z1y0", "z1y1")):
                    s = slabs[pair]
                    s3 = s[:].rearrange("p (w c) -> p w c", c=C)
                    # both channels per op, on ACT to keep DVE free for voxel math
                    nc.scalar.activation(
                        ez3[:, :, pi * 2: pi * 2 + 2], s3[:, :, :], ACTF.Copy)
                # store on the ACT HWDGE ring so load/store issue streams parallelize
                nc.scalar.dma_start(out=e_v[z], in_=ez[:])

        with tc.tile_pool(name="io", bufs=2) as io, tc.tile_pool(name="tmp", bufs=1) as tp:
            for ch in range(NCHUNK):
                j0 = ch * CK
                sl = slice(j0, j0 + CK)

                def lt(pool, name, dt=F32, cols=CK):
                    return pool.tile([P, cols], dt, tag=name, name=f"{name}_c{ch}")

                # ---- loads ----
                px, py, pz = (lt(io, f"g{c}") for c in range(3))
                for t, c in ((px, 0), (py, 1), (pz, 2)):
                    nc.sync.dma_start(out=t[:], in_=grid_v[c, :, sl])
                fl = [lt(io, f"f{c}") for c in range(7)]
                for c in range(7):
                    nc.sync.dma_start(out=fl[c][:], in_=flow_v[c, :, sl])
                tx, ty, tz, qx, qy, qz, qw = fl

                # ---- SE3: uv = qv x p + qw*p ; n = p + 2*(qv x uv) + t ----
                m1 = lt(tp, "m1"); m2 = lt(tp, "m2")
                uv = [lt(tp, f"uv{c}") for c in range(3)]
                pcomp = (px, py, pz)
                qv = (qx, qy, qz)
                for c in range(3):
                    a, b = (c + 1) % 3, (c + 2) % 3
                    nc.vector.tensor_mul(m1[:], qv[a][:], pcomp[b][:])
                    nc.vector.tensor_mul(m2[:], qv[b][:], pcomp[a][:])
                    nc.vector.tensor_sub(uv[c][:], m1[:], m2[:])
                    nc.vector.tensor_mul(m1[:], qw[:], pcomp[c][:])
                    nc.vector.tensor_add(uv[c][:], uv[c][:], m1[:])
                nlocs = [lt(io, f"n{c}") for c in range(3)]
                tcomp = (tx, ty, tz)
                for c in range(3):
                    a, b = (c + 1) % 3, (c + 2) % 3
                    nc.vector.tensor_mul(m1[:], qv[a][:], uv[b][:])
                    nc.vector.tensor_mul(m2[:], qv[b][:], uv[a][:])
                    nc.vector.tensor_sub(m1[:], m1[:], m2[:])
                    nc.vector.scalar_tensor_tensor(
                        out=m1[:], in0=m1[:], scalar=2.0, in1=pcomp[c][:],
                        op0=A.mult, op1=A.add)
                    nc.vector.tensor_add(nlocs[c][:], m1[:], tcomp[c][:])
                    nc.sync.dma_start(out=nloc_v[c, :, sl], in_=nlocs[c][:])

                # ---- sampling coords per axis: i = clip(79.5*n + c0, 0, L-1) ----
                # floor via round-to-nearest(i - 0.5)  (cast rounds nearest-even;
                # exact-odd-integer i gives floor = i-1 with w = 1.0 -> still an
                # exact, in-bounds lerp)
                cf = []; wfrac = []
                for c, (c0, L) in enumerate(((47.5, D), (63.5, H), (79.5, W))):
                    ic = lt(tp, f"ic{c}")
                    nc.vector.tensor_scalar(
                        out=ic[:], in0=nlocs[c][:], scalar1=79.5, scalar2=c0,
                        op0=A.mult, op1=A.add)
                    nc.vector.tensor_scalar(
                        out=ic[:], in0=ic[:], scalar1=0.0, scalar2=float(L - 1),
                        op0=A.max, op1=A.min)
                    ci = lt(tp, f"ci{c}", I32)
                    nc.scalar.activation(ci[:], ic[:], ACTF.Copy, bias=-0.5)
                    cfc = lt(tp, f"cf{c}")
                    nc.scalar.activation(cfc[:], ci[:], ACTF.Copy)
                    wc = lt(tp, f"w{c}")
                    nc.vector.tensor_sub(wc[:], ic[:], cfc[:])
                    cf.append(cfc); wfrac.append(wc)
                cfd, cfh, cfw = cf
                wz, wy, wx = wfrac

                # ---- cell offset e(z,y,x) = z*HW_ + y*W + x (f32-exact) ----
                e00 = lt(tp, "e00")
                nc.vector.scalar_tensor_tensor(
                    out=e00[:], in0=cfd[:], scalar=float(HW_), in1=cfw[:],
                    op0=A.mult, op1=A.add)
                nc.vector.scalar_tensor_tensor(
                    out=e00[:], in0=cfh[:], scalar=float(W), in1=e00[:],
                    op0=A.mult, op1=A.add)

                # ---- gather: one 64B fetch per voxel from E (all 8 corners x C) ----
                # walrus indirect DMA consumes ONE offset per partition per call.
                oi = lt(io, "O00", I32)
                nc.scalar.activation(oi[:], e00[:], ACTF.Copy)
                G = io.tile([P, CK, 16], F32, tag="G", name=f"G_c{ch}")
                for j in range(CK):
                    nc.gpsimd.indirect_dma_start(
                        out=G[:, j, :], out_offset=None, in_=e_t.ap(),
                        in_offset=bass.IndirectOffsetOnAxis(ap=oi[:, j:j + 1], axis=0))

                # ---- trilinear: lerp x within each corner pair, then y, then z ----
                # G floats 0..7 = E[cell] (x0 corners), 8..15 = E[cell+1] (x1 corners)
                sx = [lt(tp, f"sx{i}") for i in range(8)]   # [pair][ch]
                for gi in range(4):
                    for c in range(2):
                        s = sx[gi * 2 + c]
                        nc.vector.tensor_sub(s[:], G[:, :, 8 + gi * 2 + c], G[:, :, gi * 2 + c])
                        nc.vector.tensor_mul(s[:], s[:], wx[:])
                        nc.vector.tensor_add(s[:], s[:], G[:, :, gi * 2 + c])
                vy = [lt(tp, f"vy{i}") for i in range(4)]   # [zlev][ch]
                for zi in range(2):
                    for c in range(2):
                        a = sx[zi * 4 + c]          # (z, y0, ch)
                        b = sx[zi * 4 + 2 + c]      # (z, y1, ch)
                        v = vy[zi * 2 + c]
                        nc.vector.tensor_sub(v[:], b[:], a[:])
                        nc.vector.tensor_mul(v[:], v[:], wy[:])
                        nc.vector.tensor_add(v[:], v[:], a[:])
                for c in range(2):
                    o = lt(io, f"out{c}")
                    nc.vector.tensor_sub(o[:], vy[2 + c][:], vy[c][:])
                    nc.vector.tensor_mul(o[:], o[:], wz[:])
                    nc.vector.tensor_add(o[:], o[:], vy[c][:])
                    nc.sync.dma_start(out=warp_v[c, :, sl], in_=o[:])

    nc.compile()
    return nc


def _get_program():
    if "nc" not in _CACHE:
        _CACHE["nc"] = _build_program()
    return _CACHE["nc"]


def _make_in_maps(src, flow):
    src = np.ascontiguousarray(np.asarray(src, dtype=np.float32))
    flow = np.ascontiguousarray(np.asarray(flow, dtype=np.float32))
    assert src.shape == (1, D, H, W, C) and flow.shape == (N, 7)

    grid = _make_grid_np()                                # (3,D,H,W) f32
    pts3 = grid.reshape(3, N)                             # component-major points

    src_cells = src.reshape(N, C)
    src_pad = np.concatenate([src_cells, np.zeros((PAD, C), np.float32)], axis=0)
    flow7 = np.ascontiguousarray(flow.T)                  # (7, N)

    in_maps = []
    for k in range(NCORES):
        sl = slice(k * TPC, (k + 1) * TPC)
        in_maps.append({
            "src": src_pad,
            "flow7": np.ascontiguousarray(flow7[:, sl]),
            "grid3": np.ascontiguousarray(pts3[:, sl]),
        })
    return in_maps


def kernel(src, flow):
    nc = _get_program()
    in_maps = _make_in_maps(src, flow)
    grid = _make_grid_np()

    res = run_bass_kernel_spmd(nc, in_maps, core_ids=list(range(NCORES)))
    warped = np.concatenate([res.results[k]["warped"] for k in range(NCORES)], axis=1)
    newloc = np.concatenate([res.results[k]["newloc"] for k in range(NCORES)], axis=1)

    warped = warped.reshape(1, C, D, H, W)
    newloc = newloc.reshape(1, 3, D, H, W)
    return warped, newloc, grid[None]


# revision 18
# speedup vs baseline: 1.0512x; 1.0059x over previous
"""Trainium2 Bass kernel for NSETransformer forward (SE3 voxel flow + 3D grid_sample).

Problem: src (1,96,128,160,2) f32 volume; flow (N,7) per-voxel SE3 [t, q].
  N = 96*128*160 = 1,966,080 voxels, C = 2 channels.
Outputs (matching reference):
  warped  (1,2,96,128,160)  trilinear border-clamped sample of src at SE3-moved points
  new_loc (1,3,96,128,160)  the SE3-transformed grid points
  grid    (1,3,96,128,160)  constant normalized grid

Sharding: flattened voxel dim split evenly across 8 cores (245,760 voxels each);
src replicated to every core's HBM; no cross-core communication.

Per-core kernel: chunks of 61,440 voxels ([128 partitions x 480 cols]).
SE3 math + sampling coords + trilinear weights on DVE/ACT; the 8-corner fetch is
4 indirect-DMA gathers per voxel chunk (one per (z,y) corner pair), each
descriptor pulling 16B = cells (z,y,x0) and (z,y,x0+1) = 4 floats (2 cells x C).
"""

import numpy as np

import concourse.bass as bass
import concourse.bacc as bacc
import concourse.mybir as mybir
from concourse import tile
from concourse.bass_utils import run_bass_kernel_spmd

D, H, W = 96, 128, 160
C = 2
N = D * H * W                 # 1,966,080
NCORES = 8
TPC = N // NCORES             # 245,760 voxels per core
P = 128                       # partitions
COLS = TPC // P               # 1920 columns per partition
CK = 480                      # chunk columns
NCHUNK = COLS // CK           # 4 chunks
PAD = 8                       # extra cells at end of src copy (last x-pair read)

MX = max(D, H, W) - 1         # 159
HW_ = H * W                   # 20480 cells per z-slice
F32 = mybir.dt.float32
I32 = mybir.dt.int32

_CACHE = {}


def _make_grid_np():
    # Matches reference _make_grid (f32 throughout)
    vs = [np.arange(s, dtype=np.float32) for s in (D, H, W)]
    g = np.stack(np.meshgrid(*vs, indexing="ij")).astype(np.float32)  # (3,D,H,W)
    for i, s in enumerate((D, H, W)):
        g[i] = 2.0 * (g[i] / (s - 1) - 0.5) * (s - 1) / MX
    return g


def _build_program():
    nc = bacc.Bacc("TRN2", target_bir_lowering=False, debug=False, num_devices=NCORES)

    src_t = nc.dram_tensor("src", (N + PAD, C), F32, kind="ExternalInput")
    flow_t = nc.dram_tensor("flow7", (7, TPC), F32, kind="ExternalInput")
    grid_t = nc.dram_tensor("grid3", (3, TPC), F32, kind="ExternalInput")
    warp_t = nc.dram_tensor("warped", (C, TPC), F32, kind="ExternalOutput")
    nloc_t = nc.dram_tensor("newloc", (3, TPC), F32, kind="ExternalOutput")
    # Expanded corner volume: E[cell] = 8 floats = v(z+pz, min-clamped y+py, x, c)
    # for (pair=(pz,py), c). One 64B gather per voxel fetches entries (cell, cell+1)
    # = all 8 corners x C (the x+1 corners are simply the next entry).
    e_t = nc.dram_tensor("evol", (N + PAD, 8), F32, kind="Internal")

    flow_v = flow_t.ap().rearrange("c (p j) -> c p j", p=P)
    grid_v = grid_t.ap().rearrange("c (p j) -> c p j", p=P)
    warp_v = warp_t.ap().rearrange("c (p j) -> c p j", p=P)
    nloc_v = nloc_t.ap().rearrange("c (p j) -> c p j", p=P)
    src_v = src_t.ap()[:N].rearrange("(d h w) c -> d h (w c)", d=D, h=H)
    e_v = e_t.ap()[:N].rearrange("(d h w) k -> d h (w k)", d=D, h=H)
    e_pad_v = e_t.ap()[N:N + 2]

    A = mybir.AluOpType
    ACTF = mybir.ActivationFunctionType

    with tile.TileContext(nc) as tc:
        # ---- Phase 0: build E volume (per z-slice; partition dim = y) ----
        with tc.tile_pool(name="eb", bufs=4) as eb:
            WC = W * C

            def load_slab_pair(zz, z):
                s0 = eb.tile([P, WC], F32, tag="slab0", name=f"s0_{z}_{zz}")
                nc.sync.dma_start(out=s0[:], in_=src_v[zz])
                s1 = eb.tile([P, WC], F32, tag="slab1", name=f"s1_{z}_{zz}")
                nc.sync.dma_start(out=s1[:P - 1], in_=src_v[zz, 1:P])
                nc.sync.dma_start(out=s1[P - 1:P], in_=src_v[zz, P - 1:P])
                return s0, s1

            # pad rows N..N+1 must be finite (last cell's +1 fetch reads row N)
            zpad = eb.tile([1, 16], F32, tag="zpad", name="zpad")
            nc.vector.memset(zpad[:], 0.0)
            nc.sync.dma_start(out=e_pad_v, in_=zpad[:])

            prev = None
            for z in range(D):
                cur = prev if prev is not None else load_slab_pair(z, z)
                nxt = cur if z == D - 1 else load_slab_pair(z + 1, z)
                prev = nxt
                slabs = {"z0y0": cur[0], "z0y1": cur[1],
                         "z1y0": nxt[0], "z1y1": nxt[1]}
                ez = eb.tile([P, W * 8], F32, tag="ez", name=f"ez_{z}")
                ez3 = ez[:].rearrange("p (w k) -> p w k", k=8)
                for pi, pair in enumerate(("z0y0", "z0y1", "z1y0", "z1y1")):
                    s = slabs[pair]
                    s3 = s[:].rearrange("p (w c) -> p w c", c=C)
                    # both channels per op, on ACT to keep DVE free for voxel math
                    nc.scalar.activation(
                        ez3[:, :, pi * 2: pi * 2 + 2], s3[:, :, :], ACTF.Copy)
                # store on the ACT HWDGE ring so load/store issue streams parallelize
                nc.scalar.dma_start(out=e_v[z], in_=ez[:])

        with tc.tile_pool(name="io", bufs=2) as io, tc.tile_pool(name="tmp", bufs=1) as tp:
            for ch in range(NCHUNK):
                j0 = ch * CK
                sl = slice(j0, j0 + CK)

                def lt(pool, name, dt=F32, cols=CK):
                    return pool.tile([P, cols], dt, tag=name, name=f"{name}_c{ch}")

                # ---- loads ----
                px, py, pz = (lt(io, f"g{c}") for c in range(3))
                for t, c in ((px, 0), (py, 1), (pz, 2)):
                    nc.sync.dma_start(out=t[:], in_=grid_v[c, :, sl])
                fl = [lt(io, f"f{c}") for c in range(7)]
                for c in range(7):
                    nc.sync.dma_start(out=fl[c][:], in_=flow_v[c, :, sl])
                tx, ty, tz, qx, qy, qz, qw = fl

                # ---- SE3: uv = qv x p + qw*p ; n = p + 2*(qv x uv) + t ----
                m1 = lt(tp, "m1"); m2 = lt(tp, "m2")
                uv = [lt(tp, f"uv{c}") for c in range(3)]
                pcomp = (px, py, pz)
                qv = (qx, qy, qz)
                for c in range(3):
                    a, b = (c + 1) % 3, (c + 2) % 3
                    nc.vector.tensor_mul(m1[:], qv[a][:], pcomp[b][:])
                    nc.vector.tensor_mul(m2[:], qv[b][:], pcomp[a][:])
                    nc.vector.tensor_sub(uv[c][:], m1[:], m2[:])
                    nc.vector.tensor_mul(m1[:], qw[:], pcomp[c][:])
                    nc.vector.tensor_add(uv[c][:], uv[c][:], m1[:])
                nlocs = [lt(io, f"n{c}") for c in range(3)]
                tcomp = (tx, ty, tz)
                for c in range(3):
                    a, b = (c + 1) % 3, (c + 2) % 3
                    nc.vector.tensor_mul(m1[:], qv[a][:], uv[b][:])
                    nc.vector.tensor_mul(m2[:], qv[b][:], uv[a][:])
                    nc.vector.tensor_sub(m1[:], m1[:], m2[:])
                    nc.vector.scalar_tensor_tensor(
                        out=m1[:], in0=m1[:], scalar=2.0, in1=pcomp[c][:],
                        op0=A.mult, op1=A.add)
                    nc.vector.tensor_add(nlocs[c][:], m1[:], tcomp[c][:])
                    nc.sync.dma_start(out=nloc_v[c, :, sl], in_=nlocs[c][:])

                # ---- sampling coords per axis: i = clip(79.5*n + c0, 0, L-1) ----
                # floor via round-to-nearest(i - 0.5)  (cast rounds nearest-even;
                # exact-odd-integer i gives floor = i-1 with w = 1.0 -> still an
                # exact, in-bounds lerp)
                cf = []; wfrac = []
                for c, (c0, L) in enumerate(((47.5, D), (63.5, H), (79.5, W))):
                    ic = lt(tp, f"ic{c}")
                    nc.vector.tensor_scalar(
                        out=ic[:], in0=nlocs[c][:], scalar1=79.5, scalar2=c0,
                        op0=A.mult, op1=A.add)
                    nc.vector.tensor_scalar(
                        out=ic[:], in0=ic[:], scalar1=0.0, scalar2=float(L - 1),
                        op0=A.max, op1=A.min)
                    ci = lt(tp, f"ci{c}", I32)
                    nc.scalar.activation(ci[:], ic[:], ACTF.Copy, bias=-0.5)
                    cfc = lt(tp, f"cf{c}")
                    nc.scalar.activation(cfc[:], ci[:], ACTF.Copy)
                    wc = lt(tp, f"w{c}")
                    nc.vector.tensor_sub(wc[:], ic[:], cfc[:])
                    cf.append(cfc); wfrac.append(wc)
                cfd, cfh, cfw = cf
                wz, wy, wx = wfrac

                # ---- cell offset e(z,y,x) = z*HW_ + y*W + x (f32-exact) ----
                e00 = lt(tp, "e00")
                nc.vector.scalar_tensor_tensor(
                    out=e00[:], in0=cfd[:], scalar=float(HW_), in1=cfw[:],
                    op0=A.mult, op1=A.add)
                nc.vector.scalar_tensor_tensor(
                    out=e00[:], in0=cfh[:], scalar=float(W), in1=e00[:],
                    op0=A.mult, op1=A.add)

                # ---- gather: one 64B fetch per voxel from E (all 8 corners x C) ----
                # walrus indirect DMA consumes ONE offset per partition per call.
                oi = lt(io, "O00", I32)
                nc.scalar.activation(oi[:], e00[:], ACTF.Copy)
                G = io.tile([P, CK, 16], F32, tag="G", name=f"G_c{ch}")
                for j in range(CK):
                    nc.gpsimd.indirect_dma_start(
                        out=G[:, j, :], out_offset=None, in_=e_t.ap(),
                        in_offset=bass.IndirectOffsetOnAxis(ap=oi[:, j:j + 1], axis=0))

                # ---- trilinear: lerp x within each corner pair, then y, then z ----
                # G floats 0..7 = E[cell] (x0 corners), 8..15 = E[cell+1] (x1 corners)
                sx = [lt(tp, f"sx{i}") for i in range(8)]   # [pair][ch]
                for gi in range(4):
                    for c in range(2):
                        s = sx[gi * 2 + c]
                        nc.vector.tensor_sub(s[:], G[:, :, 8 + gi * 2 + c], G[:, :, gi * 2 + c])
                        nc.vector.tensor_mul(s[:], s[:], wx[:])
                        nc.vector.tensor_add(s[:], s[:], G[:, :, gi * 2 + c])
                vy = [lt(tp, f"vy{i}") for i in range(4)]   # [zlev][ch]
                for zi in range(2):
                    for c in range(2):
                        a = sx[zi * 4 + c]          # (z, y0, ch)
                        b = sx[zi * 4 + 2 + c]      # (z, y1, ch)
                        v = vy[zi * 2 + c]
                        nc.vector.tensor_sub(v[:], b[:], a[:])
                        nc.vector.tensor_mul(v[:], v[:], wy[:])
                        nc.vector.tensor_add(v[:], v[:], a[:])
                for c in range(2):
                    o = lt(io, f"out{c}")
                    nc.vector.tensor_sub(o[:], vy[2 + c][:], vy[c][:])
                    nc.vector.tensor_mul(o[:], o[:], wz[:])
                    nc.vector.tensor_add(o[:], o[:], vy[c][:])
                    nc.sync.dma_start(out=warp_v[c, :, sl], in_=o[:])

    nc.compile()
    return nc


def _get_program():
    if "nc" not in _CACHE:
        _CACHE["nc"] = _build_program()
    return _CACHE["nc"]


def _make_in_maps(src, flow):
    src = np.ascontiguousarray(np.asarray(src, dtype=np.float32))
    flow = np.ascontiguousarray(np.asarray(flow, dtype=np.float32))
    assert src.shape == (1, D, H, W, C) and flow.shape == (N, 7)

    grid = _make_grid_np()                                # (3,D,H,W) f32
    pts3 = grid.reshape(3, N)                             # component-major points

    src_cells = src.reshape(N, C)
    src_pad = np.concatenate([src_cells, np.zeros((PAD, C), np.float32)], axis=0)
    flow7 = np.ascontiguousarray(flow.T)                  # (7, N)

    in_maps = []
    for k in range(NCORES):
        sl = slice(k * TPC, (k + 1) * TPC)
        in_maps.append({
            "src": src_pad,
            "flow7": np.ascontiguousarray(flow7[:, sl]),
            "grid3": np.ascontiguousarray(pts3[:, sl]),
        })
    return in_maps


def kernel(src, flow):
    nc = _get_program()
    in_maps = _make_in_maps(src, flow)
    grid = _make_grid_np()

    res = run_bass_kernel_spmd(nc, in_maps, core_ids=list(range(NCORES)))
    warped = np.concatenate([res.results[k]["warped"] for k in range(NCORES)], axis=1)
    newloc = np.concatenate([res.results[k]["newloc"] for k in range(NCORES)], axis=1)

    warped = warped.reshape(1, C, D, H, W)
    newloc = newloc.reshape(1, 3, D, H, W)
    return warped, newloc, grid[None]
